# revision 7
# baseline (speedup 1.0000x reference)
"""IterSpatialCorrelationSampler (P=9, DP=1) Trainium2 Bass kernel.

out[b,i,j,y,x] = sum_c in1[b,c,y,x] * pad(in2)[b,c,y+i,x+j]   (pad=4 each side)

Strategy (v2):
  - 8 cores, each handles (b, yhalf): b = core//2, 48 rows of y.
  - TensorE Gram-band formulation: m-tile = 8y x 16x = 128 output positions
    (PSUM partitions), n = 16x24 = 384 window of padded in2 (free dim),
    contraction over c (256 = 2 accumulating matmuls of k=128).
    psum[(yt,xt), (dy,dx)] = sum_c in1[c, y0+yt, x0+xt] * in2pad[c, y0+dy, x0+dx]
    The 81 useful values per position are psum[(yt,xt), (yt+di, xt+dj)].
  - The matmul moving operand is a strided 2D window AP directly into the
    compact padded in2 SBUF tile (no window materialization copies).
  - PSUM -> SBUF evacuation alternates ACT/DVE, full 384/partition (1 op).
  - Band DMA-out is row-extracted: for partition group g (yt=g, 16
    partitions), only window rows g..g+8 (216 contiguous elems) are
    stored: 8 sliced DMAs per ty -> 2.65 MB instead of 4.72 MB.
  - Host extracts the 81 (di,dj) diagonals from the row-extracted band.
  - PE warm-up: dummy matmuls at kernel start keep the PE busy while the
    first DMAs land, flipping the HAM clock gate to 2.4 GHz early.
  - Inputs cast to fp16 on host. PSUM accumulation is fp32.
"""

import numpy as np

import concourse.bass as bass
import concourse.bacc as bacc
import concourse.tile as tile
import concourse.mybir as mybir
from concourse.bass_utils import run_bass_kernel_spmd

# problem constants (hardcoded per contract)
B, C, H, W = 4, 256, 96, 128
P = 9
OFF = 4
NCORES = 8
YH = H // 2          # 48 rows per core
WP = W + 2 * OFF     # 136
ROWS = YH + 2 * OFF  # 56 rows of padded in2 per core
MT_Y, MT_X = 8, 16   # m-tile shape (8y x 16x = 128 partitions)
NW_Y, NW_X = MT_Y + P - 1, MT_X + P - 1   # 16 x 24 window
NTY, NTX = YH // MT_Y, W // MT_X          # 6 x 8 = 48 tiles
NFREE = NW_Y * NW_X                       # 384
RE = P * NW_X                             # 216 row-extracted elems/partition
NWARM = 9                                 # PE warm-up dummy matmuls

_cached = {}


def _build():
    nc = bacc.Bacc(
        "TRN2",
        target_bir_lowering=False,
        debug=False,
        enable_asserts=False,
        num_devices=NCORES,
    )
    f16 = mybir.dt.float16
    f32 = mybir.dt.float32

    in1_d = nc.dram_tensor(
        "in1t", [128, NTY, NTX, 2, MT_Y * MT_X], f16, kind="ExternalInput"
    ).ap()
    in2_d = nc.dram_tensor("in2c", [128, 2, ROWS, WP], f16, kind="ExternalInput").ap()
    # [ty, g, lp, di(9 rows), tx, wx] — per (ty,g) slice is contiguous
    band_d = nc.dram_tensor(
        "rband", [NTY, MT_Y, NW_Y, P, NTX, NW_X], f16, kind="ExternalOutput"
    ).ap()

    with tile.TileContext(nc) as tc:
        with (
            tc.tile_pool(name="sb2", bufs=1) as sb2,
            tc.tile_pool(name="ld", bufs=3) as ld,
            tc.tile_pool(name="stage", bufs=3) as stage,
            tc.tile_pool(name="warm", bufs=1) as warm,
            tc.tile_pool(name="ps", bufs=7, space="PSUM") as ps,
            tc.tile_pool(name="psw", bufs=1, space="PSUM") as psw,
        ):
            # PE warm-up: dummy matmuls on a zero scratch tile keep the PE
            # active while input DMAs land (HAM flips to 2.4 GHz after
            # ~3.4us of sustained activity).
            ws = warm.tile([128, 512], f16)
            nc.vector.memset(ws[:, :], 0.0)
            wp = psw.tile([128, 512], f32)
            for _ in range(NWARM):
                nc.tensor.matmul(
                    wp[:, :], ws[:, 0:128], ws[:, :], start=True, stop=True
                )

            in2_sb = sb2.tile([128, 2, ROWS, WP], f16)
            # split the compact in2 load so ty=0 can start early
            nc.sync.dma_start(out=in2_sb[:, :, 0:16, :], in_=in2_d[:, :, 0:16, :])
            nc.sync.dma_start(out=in2_sb[:, :, 16:32, :], in_=in2_d[:, :, 16:32, :])
            nc.sync.dma_start(out=in2_sb[:, :, 32:ROWS, :], in_=in2_d[:, :, 32:ROWS, :])

            for ty in range(NTY):
                in1_c = ld.tile([128, NTX, 2, MT_Y * MT_X], f16, tag="in1c")
                nc.sync.dma_start(out=in1_c[:, :, :, :], in_=in1_d[:, ty, :, :, :])
                # band staged per partition as [wy, tx, wx] so that the
                # row-extracted slice (rows g..g+8, all tx) is contiguous
                bs = stage.tile([128, NW_Y, NTX, NW_X], f16, tag="bs")
                for tx in range(NTX):
                    pt = ps.tile([128, NW_Y, NW_X], f32, tag="pt")
                    for ch in range(2):
                        nc.tensor.matmul(
                            pt[:, :, :],
                            in1_c[:, tx, ch, :],
                            in2_sb[
                                :, ch,
                                MT_Y * ty : MT_Y * ty + NW_Y,
                                MT_X * tx : MT_X * tx + NW_X,
                            ],
                            start=(ch == 0),
                            stop=(ch == 1),
                        )
                    if tx % 2 == 0:
                        nc.scalar.mul(bs[:, :, tx, :], pt[:, :, :], 1.0)
                    else:
                        nc.vector.tensor_copy(bs[:, :, tx, :], pt[:, :, :])
                # row-extracted band store: group g (yt=g) keeps window rows
                # g..g+8 over all tx = 1728 contiguous elems per partition
                for g in range(MT_Y):
                    eng = nc.scalar if g % 2 == 0 else nc.sync
                    eng.dma_start(
                        out=band_d[ty, g, :, :, :, :],
                        in_=bs[g * 16 : (g + 1) * 16, g : g + P, :, :],
                    )

    nc.compile()
    return nc


def _prep_inputs(input1, input2):
    """Build per-core input maps (fp16, padded, tiled, c split on partitions)."""
    in_maps = []
    pad2 = np.pad(
        np.asarray(input2), ((0, 0), (0, 0), (OFF, OFF), (OFF, OFF))
    )  # [B, C, H+8, WP]
    a1 = np.asarray(input1)
    for core in range(NCORES):
        b, yh = core // 2, core % 2
        y0 = yh * YH
        # in1 tiles: [cp, ty, tx, ch, (my, mx)]
        i1 = a1[b, :, y0 : y0 + YH, :].reshape(2, 128, NTY, MT_Y, NTX, MT_X)
        i1 = i1.transpose(1, 2, 4, 0, 3, 5).reshape(128, NTY, NTX, 2, MT_Y * MT_X)
        # compact padded in2: [cp, ch, rows, cols]
        p2 = pad2[b, :, y0 : y0 + ROWS, :].reshape(2, 128, ROWS, WP)
        i2c = p2.transpose(1, 0, 2, 3).astype(np.float16)  # [128, 2, ROWS, WP]
        in_maps.append(
            {
                "in1t": np.ascontiguousarray(i1.astype(np.float16)),
                "in2c": np.ascontiguousarray(i2c),
            }
        )
    return in_maps


def _extract(rb):
    """rband [NTY, MT_Y, NW_Y, P, NTX, NW_X] f16 -> out_local [9, 9, 48, 128].

    rb[ty, g, lp, di, tx, wx] = band value at window row (g+di), col wx
    for position (y = ty*8+g, x = tx*16+lp). Useful wx = lp + dj.
    """
    out = np.empty((P, P, YH, W), dtype=np.float32)
    for di in range(P):
        t = rb[:, :, :, di, :, :]  # [ty, g, lp, tx, wx]
        for dj in range(P):
            d = t.diagonal(dj, 2, 4)  # [ty, g, tx, lp(diag)]
            out[di, dj] = d.reshape(YH, W)
    return out


def run(input1, input2, trace=False, **trace_kwargs):
    if "nc" not in _cached:
        _cached["nc"] = _build()
    nc = _cached["nc"]
    in_maps = _prep_inputs(input1, input2)
    res = run_bass_kernel_spmd(
        nc, in_maps, list(range(NCORES)), trace=trace, **trace_kwargs
    )
    out = np.empty((B, P, P, H, W), dtype=np.float32)
    for core in range(NCORES):
        b, yh = core // 2, core % 2
        rb = res.results[core]["rband"]
        out[b, :, :, yh * YH : (yh + 1) * YH, :] = _extract(rb)
    return out, res


def kernel(input1, input2):
    out, _ = run(input1, input2, trace=False)
    return out


# revision 13
# speedup vs baseline: 1.2234x; 1.2234x over previous
"""IterSpatialCorrelationSampler (P=9, DP=1) Trainium2 Bass kernel.

out[b,i,j,y,x] = sum_c in1[b,c,y,x] * pad(in2)[b,c,y+i,x+j]   (pad=4 each side)

Strategy (v2):
  - 8 cores, each handles (b, yhalf): b = core//2, 48 rows of y.
  - TensorE Gram-band formulation: m-tile = 8y x 16x = 128 output positions
    (PSUM partitions), n = 16x24 = 384 window of padded in2 (free dim),
    contraction over c (256 = 2 accumulating matmuls of k=128).
    psum[(yt,xt), (dy,dx)] = sum_c in1[c, y0+yt, x0+xt] * in2pad[c, y0+dy, x0+dx]
    The 81 useful values per position are psum[(yt,xt), (yt+di, xt+dj)].
  - The matmul moving operand is a strided 2D window AP directly into the
    compact padded in2 SBUF tile (no window materialization copies).
  - PSUM -> SBUF evacuation alternates ACT/DVE, full 384/partition (1 op).
  - Band DMA-out is row-extracted: for partition group g (yt=g, 16
    partitions), only window rows g..g+8 (216 contiguous elems) are
    stored: 8 sliced DMAs per ty -> 2.65 MB instead of 4.72 MB.
  - Host extracts the 81 (di,dj) diagonals from the row-extracted band.
  - PE warm-up: dummy matmuls at kernel start keep the PE busy while the
    first DMAs land, flipping the HAM clock gate to 2.4 GHz early.
  - Inputs cast to fp16 on host. PSUM accumulation is fp32.
"""

import numpy as np

import concourse.bass as bass
import concourse.bacc as bacc
import concourse.tile as tile
import concourse.mybir as mybir
from concourse.bass_utils import run_bass_kernel_spmd

# problem constants (hardcoded per contract)
B, C, H, W = 4, 256, 96, 128
P = 9
OFF = 4
NCORES = 8
YH = H // 2          # 48 rows per core
WP = W + 2 * OFF     # 136
ROWS = YH + 2 * OFF  # 56 rows of padded in2 per core
MT_Y, MT_X = 8, 16   # m-tile shape (8y x 16x = 128 partitions)
NW_Y, NW_X = MT_Y + P - 1, MT_X + P - 1   # 16 x 24 window
NTY, NTX = YH // MT_Y, W // MT_X          # 6 x 8 = 48 tiles
NFREE = NW_Y * NW_X                       # 384
RE = P * NW_X                             # 216 row-extracted elems/partition
NWARM = 8                                 # PE warm-up dummy matmuls

_cached = {}


def _build():
    nc = bacc.Bacc(
        "TRN2",
        target_bir_lowering=False,
        debug=False,
        enable_asserts=False,
        num_devices=NCORES,
    )
    f16 = mybir.dt.float16
    f32 = mybir.dt.float32

    in1_d = nc.dram_tensor(
        "in1t", [128, NTY, NTX, 2, MT_Y * MT_X], f16, kind="ExternalInput"
    ).ap()
    in2_d = nc.dram_tensor("in2c", [128, 2, ROWS, WP], f16, kind="ExternalInput").ap()
    # [h, g, lp, ty3, di(9 rows), tx, wx] — per (h,g) slice is contiguous
    TYH = NTY // 2
    band_d = nc.dram_tensor(
        "rband", [2, MT_Y, NW_Y, TYH, P, NTX, NW_X], f16, kind="ExternalOutput"
    ).ap()

    with tile.TileContext(nc) as tc:
        with (
            tc.tile_pool(name="sb2", bufs=1) as sb2,
            tc.tile_pool(name="ld", bufs=3) as ld,
            tc.tile_pool(name="stage", bufs=3) as stage,
            tc.tile_pool(name="warm", bufs=1) as warm,
            tc.tile_pool(name="ps", bufs=7, space="PSUM") as ps,
            tc.tile_pool(name="psw", bufs=1, space="PSUM") as psw,
        ):
            # PE warm-up: dummy matmuls on a zero scratch tile keep the PE
            # active while input DMAs land (HAM flips to 2.4 GHz after
            # ~3.4us of sustained activity).
            ws = warm.tile([128, 512], f16)
            nc.vector.memset(ws[:, :], 0.0)
            wp = psw.tile([128, 512], f32)
            for _ in range(NWARM):
                nc.tensor.matmul(
                    wp[:, :], ws[:, 0:128], ws[:, :], start=True, stop=True
                )

            in2_sb = sb2.tile([128, 2, ROWS, WP], f16)
            # whole-run band staging buffer: [p, ty, wy, tx, wx]; the
            # row-extracted slice (rows g..g+8, all tx, a ty-range) is a
            # 3-dim DMA AP with 1728-elem contiguous runs
            bs = sb2.tile([128, NTY, NW_Y, NTX, NW_X], f16)
            in1_cs = [
                ld.tile([128, NTX, 2, MT_Y * MT_X], f16, tag="in1c", name=f"in1c{i}")
                for i in range(2)
            ]
            # load priority: first compute tile's deps first (ty0+chunk0),
            # then the rest (issue order staggers SDMA round-robin starts)
            nc.sync.dma_start(out=in2_sb[:, :, 0:16, :], in_=in2_d[:, :, 0:16, :])
            nc.sync.dma_start(out=in1_cs[0][:, :, :, :], in_=in1_d[:, 0, :, :, :])
            nc.sync.dma_start(out=in1_cs[1][:, :, :, :], in_=in1_d[:, 1, :, :, :])
            nc.sync.dma_start(out=in2_sb[:, :, 16:32, :], in_=in2_d[:, :, 16:32, :])
            nc.sync.dma_start(out=in2_sb[:, :, 32:ROWS, :], in_=in2_d[:, :, 32:ROWS, :])

            for ty in range(NTY):
                if ty >= 2:
                    in1_c = ld.tile([128, NTX, 2, MT_Y * MT_X], f16, tag="in1c")
                    nc.sync.dma_start(out=in1_c[:, :, :, :], in_=in1_d[:, ty, :, :, :])
                else:
                    in1_c = in1_cs[ty]
                for tx in range(NTX):
                    pt = ps.tile([128, NW_Y, NW_X], f32, tag="pt")
                    for ch in range(2):
                        nc.tensor.matmul(
                            pt[:, :, :],
                            in1_c[:, tx, ch, :],
                            in2_sb[
                                :, ch,
                                MT_Y * ty : MT_Y * ty + NW_Y,
                                MT_X * tx : MT_X * tx + NW_X,
                            ],
                            start=(ch == 0),
                            stop=(ch == 1),
                        )
                    if tx % 2 == 0:
                        nc.scalar.mul(bs[:, ty, :, tx, :], pt[:, :, :], 1.0)
                    else:
                        nc.vector.tensor_copy(bs[:, ty, :, tx, :], pt[:, :, :])
                # after each ty-half completes, store its row-extracted band:
                # one DMA per group g covers 3 ty at once
                if ty % TYH == TYH - 1:
                    h = ty // TYH
                    for g in range(MT_Y):
                        eng = nc.scalar if g % 2 == 0 else nc.sync
                        eng.dma_start(
                            out=band_d[h, g, :, :, :, :, :],
                            in_=bs[
                                g * 16 : (g + 1) * 16,
                                h * TYH : (h + 1) * TYH,
                                g : g + P,
                                :,
                                :,
                            ],
                        )

    nc.compile()
    return nc


def _prep_inputs(input1, input2):
    """Build per-core input maps (fp16, padded, tiled, c split on partitions)."""
    in_maps = []
    pad2 = np.pad(
        np.asarray(input2), ((0, 0), (0, 0), (OFF, OFF), (OFF, OFF))
    )  # [B, C, H+8, WP]
    a1 = np.asarray(input1)
    for core in range(NCORES):
        b, yh = core // 2, core % 2
        y0 = yh * YH
        # in1 tiles: [cp, ty, tx, ch, (my, mx)]
        i1 = a1[b, :, y0 : y0 + YH, :].reshape(2, 128, NTY, MT_Y, NTX, MT_X)
        i1 = i1.transpose(1, 2, 4, 0, 3, 5).reshape(128, NTY, NTX, 2, MT_Y * MT_X)
        # compact padded in2: [cp, ch, rows, cols]
        p2 = pad2[b, :, y0 : y0 + ROWS, :].reshape(2, 128, ROWS, WP)
        i2c = p2.transpose(1, 0, 2, 3).astype(np.float16)  # [128, 2, ROWS, WP]
        in_maps.append(
            {
                "in1t": np.ascontiguousarray(i1.astype(np.float16)),
                "in2c": np.ascontiguousarray(i2c),
            }
        )
    return in_maps


def _extract(rb):
    """rband [2, MT_Y, NW_Y, NTY//2, P, NTX, NW_X] f16 -> [9, 9, 48, 128].

    rb[h, g, lp, ty3, di, tx, wx] = band value at window row (g+di), col wx
    for position (y = (h*3+ty3)*8+g, x = tx*16+lp). Useful wx = lp + dj.
    """
    # -> [ty(6), g, lp, di, tx, wx]
    arr = rb.transpose(0, 3, 1, 2, 4, 5, 6).reshape(NTY, MT_Y, NW_Y, P, NTX, NW_X)
    out = np.empty((P, P, YH, W), dtype=np.float32)
    for di in range(P):
        t = arr[:, :, :, di, :, :]  # [ty, g, lp, tx, wx]
        for dj in range(P):
            d = t.diagonal(dj, 2, 4)  # [ty, g, tx, lp(diag)]
            out[di, dj] = d.reshape(YH, W)
    return out


def run(input1, input2, trace=False, **trace_kwargs):
    if "nc" not in _cached:
        _cached["nc"] = _build()
    nc = _cached["nc"]
    in_maps = _prep_inputs(input1, input2)
    res = run_bass_kernel_spmd(
        nc, in_maps, list(range(NCORES)), trace=trace, **trace_kwargs
    )
    out = np.empty((B, P, P, H, W), dtype=np.float32)
    for core in range(NCORES):
        b, yh = core // 2, core % 2
        rb = res.results[core]["rband"]
        out[b, :, :, yh * YH : (yh + 1) * YH, :] = _extract(rb)
    return out, res


def kernel(input1, input2):
    out, _ = run(input1, input2, trace=False)
    return out


# revision 18
# speedup vs baseline: 1.3192x; 1.0782x over previous
"""IterSpatialCorrelationSampler (P=9, DP=1) Trainium2 Bass kernel.

out[b,i,j,y,x] = sum_c in1[b,c,y,x] * pad(in2)[b,c,y+i,x+j]   (pad=4 each side)

Strategy (v2):
  - 8 cores, each handles (b, yhalf): b = core//2, 48 rows of y.
  - TensorE Gram-band formulation: m-tile = 8y x 16x = 128 output positions
    (PSUM partitions), n = 16x24 = 384 window of padded in2 (free dim),
    contraction over c (256 = 2 accumulating matmuls of k=128).
    psum[(yt,xt), (dy,dx)] = sum_c in1[c, y0+yt, x0+xt] * in2pad[c, y0+dy, x0+dx]
    The 81 useful values per position are psum[(yt,xt), (yt+di, xt+dj)].
  - The matmul moving operand is a strided 2D window AP directly into the
    compact padded in2 SBUF tile (no window materialization copies).
  - PSUM -> SBUF evacuation alternates ACT/DVE, full 384/partition (1 op).
  - Band DMA-out is row-extracted: for partition group g (yt=g, 16
    partitions), only window rows g..g+8 (216 contiguous elems) are
    stored: 8 sliced DMAs per ty -> 2.65 MB instead of 4.72 MB.
  - Host extracts the 81 (di,dj) diagonals from the row-extracted band.
  - PE warm-up: dummy matmuls at kernel start keep the PE busy while the
    first DMAs land, flipping the HAM clock gate to 2.4 GHz early.
  - Inputs cast to fp16 on host. PSUM accumulation is fp32.
"""

import numpy as np

import concourse.bass as bass
import concourse.bacc as bacc
import concourse.tile as tile
import concourse.mybir as mybir
from concourse.bass_utils import run_bass_kernel_spmd

# problem constants (hardcoded per contract)
B, C, H, W = 4, 256, 96, 128
P = 9
OFF = 4
NCORES = 8
YH = H // 2          # 48 rows per core
WP = W + 2 * OFF     # 136
ROWS = YH + 2 * OFF  # 56 rows of padded in2 per core
MT_Y, MT_X = 8, 16   # m-tile shape (8y x 16x = 128 partitions)
NW_Y, NW_X = MT_Y + P - 1, MT_X + P - 1   # 16 x 24 window
NTY, NTX = YH // MT_Y, W // MT_X          # 6 x 8 = 48 tiles
NFREE = NW_Y * NW_X                       # 384
RE = P * NW_X                             # 216 row-extracted elems/partition
NWARM = 16                                # PE warm-up dummy matmuls
NWAVE = 3                                 # band store waves
TYW = NTY // NWAVE                        # ty rows per store wave

_cached = {}


def _build():
    nc = bacc.Bacc(
        "TRN2",
        target_bir_lowering=False,
        debug=False,
        enable_asserts=False,
        num_devices=NCORES,
    )
    f16 = mybir.dt.float16
    f32 = mybir.dt.float32

    in1_d = nc.dram_tensor(
        "in1t", [128, NTY, NTX, 2, MT_Y * MT_X], f16, kind="ExternalInput"
    ).ap()
    in2_d = nc.dram_tensor("in2c", [128, 2, ROWS, WP], f16, kind="ExternalInput").ap()
    # [w, g, lp, tyw, di(9 rows), tx, wx] — per (w,g) slice is contiguous
    band_d = nc.dram_tensor(
        "rband", [NWAVE, MT_Y, NW_Y, TYW, P, NTX, NW_X], f16, kind="ExternalOutput"
    ).ap()

    with tile.TileContext(nc) as tc:
        with (
            tc.tile_pool(name="sb2", bufs=1) as sb2,
            tc.tile_pool(name="ld", bufs=4) as ld,
            tc.tile_pool(name="stage", bufs=3) as stage,
            tc.tile_pool(name="warm", bufs=1) as warm,
            tc.tile_pool(name="ps", bufs=7, space="PSUM") as ps,
            tc.tile_pool(name="psw", bufs=1, space="PSUM") as psw,
        ):
            # PE warm-up: dummy matmuls on a zero scratch tile keep the PE
            # active while input DMAs land (HAM flips to 2.4 GHz after
            # ~3.4us of sustained activity).
            ws = warm.tile([128, 512], f16)
            nc.vector.memset(ws[:, :], 0.0)
            wp = psw.tile([128, 512], f32)
            for _ in range(NWARM):
                nc.tensor.matmul(
                    wp[:, :], ws[:, 0:128], ws[:, :], start=True, stop=True
                )

            in2_sb = sb2.tile([128, 2, ROWS, WP], f16)
            # whole-run band staging buffer: [p, ty, wy, tx, wx]; the
            # row-extracted slice (rows g..g+8, all tx, a ty-range) is a
            # 3-dim DMA AP with 1728-elem contiguous runs
            bs = sb2.tile([128, NTY, NW_Y, NTX, NW_X], f16)
            in1_cs = [
                ld.tile([128, NTX, 2, MT_Y * MT_X], f16, tag="in1c", name=f"in1c{i}")
                for i in range(2)
            ]
            # load priority: first compute tile's deps first (ty0+chunk0),
            # then the rest (issue order staggers SDMA round-robin starts)
            nc.sync.dma_start(out=in2_sb[:, :, 0:16, :], in_=in2_d[:, :, 0:16, :])
            nc.sync.dma_start(out=in1_cs[0][:, :, :, :], in_=in1_d[:, 0, :, :, :])
            nc.sync.dma_start(out=in1_cs[1][:, :, :, :], in_=in1_d[:, 1, :, :, :])
            nc.sync.dma_start(out=in2_sb[:, :, 16:32, :], in_=in2_d[:, :, 16:32, :])
            nc.sync.dma_start(out=in2_sb[:, :, 32:ROWS, :], in_=in2_d[:, :, 32:ROWS, :])

            for ty in range(NTY):
                if ty >= 2:
                    in1_c = ld.tile([128, NTX, 2, MT_Y * MT_X], f16, tag="in1c")
                    nc.sync.dma_start(out=in1_c[:, :, :, :], in_=in1_d[:, ty, :, :, :])
                else:
                    in1_c = in1_cs[ty]
                for tx in range(NTX):
                    pt = ps.tile([128, NW_Y, NW_X], f32, tag="pt")
                    for ch in range(2):
                        nc.tensor.matmul(
                            pt[:, :, :],
                            in1_c[:, tx, ch, :],
                            in2_sb[
                                :, ch,
                                MT_Y * ty : MT_Y * ty + NW_Y,
                                MT_X * tx : MT_X * tx + NW_X,
                            ],
                            start=(ch == 0),
                            stop=(ch == 1),
                        )
                    if tx % 2 == 0:
                        nc.scalar.mul(bs[:, ty, :, tx, :], pt[:, :, :], 1.0)
                    else:
                        nc.vector.tensor_copy(bs[:, ty, :, tx, :], pt[:, :, :])
                # after each ty-pair completes, store its row-extracted band:
                # one DMA per group g covers TYW ty rows at once. Early waves
                # issue from the otherwise-idle GPSIMD (SWDGE); the last wave
                # uses the fast HWDGE engines to shorten the drain tail.
                if ty % TYW == TYW - 1:
                    w = ty // TYW
                    for g in range(MT_Y):
                        if w < NWAVE - 1:
                            eng = nc.gpsimd
                        else:
                            eng = nc.scalar if g % 2 == 0 else nc.sync
                        eng.dma_start(
                            out=band_d[w, g, :, :, :, :, :],
                            in_=bs[
                                g * 16 : (g + 1) * 16,
                                w * TYW : (w + 1) * TYW,
                                g : g + P,
                                :,
                                :,
                            ],
                        )

    nc.compile()
    return nc


def _prep_inputs(input1, input2):
    """Build per-core input maps (fp16, padded, tiled, c split on partitions)."""
    in_maps = []
    pad2 = np.pad(
        np.asarray(input2), ((0, 0), (0, 0), (OFF, OFF), (OFF, OFF))
    )  # [B, C, H+8, WP]
    a1 = np.asarray(input1)
    for core in range(NCORES):
        b, yh = core // 2, core % 2
        y0 = yh * YH
        # in1 tiles: [cp, ty, tx, ch, (my, mx)]
        i1 = a1[b, :, y0 : y0 + YH, :].reshape(2, 128, NTY, MT_Y, NTX, MT_X)
        i1 = i1.transpose(1, 2, 4, 0, 3, 5).reshape(128, NTY, NTX, 2, MT_Y * MT_X)
        # compact padded in2: [cp, ch, rows, cols]
        p2 = pad2[b, :, y0 : y0 + ROWS, :].reshape(2, 128, ROWS, WP)
        i2c = p2.transpose(1, 0, 2, 3).astype(np.float16)  # [128, 2, ROWS, WP]
        in_maps.append(
            {
                "in1t": np.ascontiguousarray(i1.astype(np.float16)),
                "in2c": np.ascontiguousarray(i2c),
            }
        )
    return in_maps


def _extract(rb):
    """rband [NWAVE, MT_Y, NW_Y, TYW, P, NTX, NW_X] f16 -> [9, 9, 48, 128].

    rb[w, g, lp, tyw, di, tx, wx] = band value at window row (g+di), col wx
    for position (y = (w*TYW+tyw)*8+g, x = tx*16+lp). Useful wx = lp + dj.
    """
    # -> [ty(6), g, lp, di, tx, wx]
    arr = rb.transpose(0, 3, 1, 2, 4, 5, 6).reshape(NTY, MT_Y, NW_Y, P, NTX, NW_X)
    out = np.empty((P, P, YH, W), dtype=np.float32)
    for di in range(P):
        t = arr[:, :, :, di, :, :]  # [ty, g, lp, tx, wx]
        for dj in range(P):
            d = t.diagonal(dj, 2, 4)  # [ty, g, tx, lp(diag)]
            out[di, dj] = d.reshape(YH, W)
    return out


def run(input1, input2, trace=False, **trace_kwargs):
    if "nc" not in _cached:
        _cached["nc"] = _build()
    nc = _cached["nc"]
    in_maps = _prep_inputs(input1, input2)
    res = run_bass_kernel_spmd(
        nc, in_maps, list(range(NCORES)), trace=trace, **trace_kwargs
    )
    out = np.empty((B, P, P, H, W), dtype=np.float32)
    for core in range(NCORES):
        b, yh = core // 2, core % 2
        rb = res.results[core]["rband"]
        out[b, :, :, yh * YH : (yh + 1) * YH, :] = _extract(rb)
    return out, res


def kernel(input1, input2):
    out, _ = run(input1, input2, trace=False)
    return out
